# revision 1
# baseline (speedup 1.0000x reference)
"""Trainium2 Bass kernel for CriticalBrainDynamics (leaky integrate-and-fire
network with global refractory coupling), SPMD over 8 NeuronCores.

Sharding: neurons (columns) sharded 512/core; batch replicated per column.
Device layout is transposed ([neuron, batch]) so that:
  - per-neuron params (thresholds, refractory) are per-partition scalars,
  - any(mask, axis=batch) is a free-axis reduction (local, no all-reduce),
  - the spike mask is produced directly in the [K, B] layout the next
    step's matmul rhs needs (no transposes on device).

Per step, spikes are exchanged with one fp8 AllGather ([513, 1024] per rank:
512 spike rows + 1 flag row carrying the local any-spike bit). A register
loaded from the gathered flag rows gates the matmul block with tc.If, so
steps where no neuron spiked anywhere skip the matmul entirely (the network
is refractory-silent 2 of every 3 steps for this regime).

The matmul is exact: connectivity and spikes are 0/1 (exact in fp8e4m3),
accumulated in fp32 PSUM, so v evolves bit-identically to the f32 reference.
"""

import numpy as np
import ml_dtypes

import concourse.bacc as bacc
import concourse.mybir as mybir
import concourse.tile as tile
from concourse.bass_utils import run_bass_kernel_spmd

N = 4096          # neurons
B = 1024          # batch
N_STEPS = 10
N_CORES = 8
J_OWN = N // N_CORES      # 512 neurons owned per core
T_TILES = J_OWN // 128    # 4 partition tiles of own neurons
K_TILES = N // 128        # 32 contraction tiles
B_BLKS = B // 512         # 2 psum free-dim blocks

F32 = mybir.dt.float32
FP8 = mybir.dt.float8e4
AOT = mybir.AluOpType

_CACHE = {}


def build_nc():
    nc = bacc.Bacc("TRN2", target_bir_lowering=False, debug=False,
                   num_devices=N_CORES)

    ext_in = nc.dram_tensor("ext_t", [J_OWN, B], F32, kind="ExternalInput")
    c_in = nc.dram_tensor("c_fp8", [N, J_OWN], FP8, kind="ExternalInput")
    mp_in = nc.dram_tensor("mp", [128, T_TILES], F32, kind="ExternalInput")
    th_in = nc.dram_tensor("th", [128, T_TILES], F32, kind="ExternalInput")
    rf_in = nc.dram_tensor("refr0", [128, T_TILES], F32, kind="ExternalInput")
    s_out = nc.dram_tensor("s_out", [J_OWN, B], F32, kind="ExternalOutput")

    with tile.TileContext(nc) as tc:
        with (
            tc.tile_pool(name="sbuf", bufs=1) as pool,
            tc.tile_pool(name="psum", bufs=6, space="PSUM") as pp,
            tc.tile_pool(name="psum2", bufs=2, space="PSUM") as pp2,
            tc.tile_pool(name="dram", bufs=2, space="DRAM") as dp,
        ):
            # --- persistent SBUF state ---
            c_sb = pool.tile([128, K_TILES * J_OWN], FP8)     # connectivity slice
            s_sb = pool.tile([128, K_TILES * B], FP8)         # gathered spikes^T
            v = pool.tile([128, T_TILES * B], F32)            # membrane v^T
            mask8 = pool.tile([128, T_TILES * B], FP8)        # spike mask^T fp8
            mask32 = pool.tile([128, T_TILES * B], F32)       # final-step mask f32
            th = pool.tile([128, T_TILES], F32)
            refr = pool.tile([128, T_TILES], F32)
            elig = pool.tile([128, T_TILES], F32)             # refr == 0
            counts = pool.tile([128, T_TILES], F32)           # spike count / partition
            anyv = pool.tile([128, T_TILES], mybir.dt.int32)
            three = pool.tile([128, T_TILES], F32)
            ones8 = pool.tile([128, 1], FP8)
            la = pool.tile([1, 1], F32)
            la8 = pool.tile([1, 1], FP8)
            fl8 = pool.tile([1, N_CORES], FP8)
            flm = pool.tile([1, 1], F32)
            fli = pool.tile([1, 1], mybir.dt.int32)
            mp_sb = pool.tile([128, T_TILES], F32)

            # --- load constants / initial state ---
            nc.sync.dma_start(
                c_sb[:].rearrange("p (k j) -> p k j", k=K_TILES),
                c_in.ap().rearrange("(k p) j -> p k j", p=128),
            )
            nc.sync.dma_start(th[:], th_in.ap())
            nc.sync.dma_start(refr[:], rf_in.ap())
            nc.sync.dma_start(mp_sb[:], mp_in.ap())
            nc.sync.dma_start(
                v[:].rearrange("p (t b) -> p t b", t=T_TILES),
                ext_in.ap().rearrange("(t p) b -> p t b", p=128),
            )
            nc.gpsimd.memset(three[:], 3.0)
            nc.gpsimd.memset(ones8[:], 1.0)
            # v0 = ext + membrane_potentials (per-partition add per t-tile)
            for t in range(T_TILES):
                nc.vector.tensor_scalar_add(
                    v[:, t * B:(t + 1) * B], v[:, t * B:(t + 1) * B],
                    mp_sb[:, t:t + 1])
            nc.vector.tensor_scalar(
                out=elig[:], in0=refr[:], scalar1=0.0, scalar2=None,
                op0=AOT.is_equal)

            any_sv = None  # ScalarValue: global any-spike of previous step

            for step in range(1, N_STEPS + 1):
                last = step == N_STEPS

                # --- network input: v += 0.1 * (s_prev @ C); skipped when
                # the previous step had no spikes anywhere (s_prev == 0).
                if step >= 2:
                    c3 = c_sb[:].rearrange("p (k j) -> p k j", k=K_TILES)
                    s3 = s_sb[:].rearrange("p (k b) -> p k b", k=K_TILES)
                    with tc.If(any_sv > 0):
                        for t in range(T_TILES):
                            for bb in range(B_BLKS):
                                ps = pp.tile([128, 512], F32, tag="ps")
                                for kp in range(K_TILES // 2):
                                    # DoubleRow: one MM = two k-tiles (2 fp8
                                    # MACs/cell/cycle; ~1.8x even HAM-cold)
                                    nc.tensor.matmul(
                                        ps[:],
                                        c3[:, 2 * kp:2 * kp + 2,
                                           t * 128:(t + 1) * 128],
                                        s3[:, 2 * kp:2 * kp + 2,
                                           bb * 512:(bb + 1) * 512],
                                        start=(kp == 0),
                                        stop=(kp == K_TILES // 2 - 1),
                                        perf_mode=mybir.MatmulPerfMode.DoubleRow,
                                    )
                                vs = v[:, t * B + bb * 512: t * B + (bb + 1) * 512]
                                nc.vector.scalar_tensor_tensor(
                                    out=vs, in0=ps[:], scalar=0.1, in1=vs,
                                    op0=AOT.mult, op1=AOT.add)

                # --- spike mask: mask = (v > th) * elig
                mout = mask32 if last else mask8
                for t in range(T_TILES):
                    nc.vector.tensor_scalar(
                        out=mout[:, t * B:(t + 1) * B],
                        in0=v[:, t * B:(t + 1) * B],
                        scalar1=th[:, t:t + 1], scalar2=elig[:, t:t + 1],
                        op0=AOT.is_gt, op1=AOT.mult)

                if last:
                    # output spikes of step 10; no gather needed
                    nc.sync.dma_start(
                        s_out.ap().rearrange("(t p) b -> p t b", p=128),
                        mask32[:].rearrange("p (t b) -> p t b", t=T_TILES))
                    break

                # --- share spikes: AllGather [513, 1024] fp8 per rank
                ag_in = dp.tile([J_OWN + 1, B], FP8, tag="agin")
                ag_out = dp.tile([(J_OWN + 1) * N_CORES, B], FP8,
                                 addr_space="Shared", tag="agout")
                nc.sync.dma_start(
                    ag_in[0:J_OWN, :].rearrange("(t p) b -> p t b", p=128),
                    mask8[:].rearrange("p (t b) -> p t b", t=T_TILES))
                # local any-spike flag -> row 512, col 0.  Collapse the whole
                # mask on the idle PE (ones-matmul over mask8 slices), so the
                # flag does not wait for the DVE counts reduction.
                la_ps = pp2.tile([1, 512], F32, tag="laps")
                for sl in range(T_TILES * B // 512):
                    nc.tensor.matmul(la_ps[:], ones8[:],
                                     mask8[:, sl * 512:(sl + 1) * 512],
                                     start=(sl == 0),
                                     stop=(sl == T_TILES * B // 512 - 1))
                nc.vector.tensor_reduce(
                    out=la[:], in_=la_ps[:], axis=mybir.AxisListType.X,
                    op=AOT.max)
                nc.vector.tensor_scalar(
                    out=la8[:], in0=la[:], scalar1=0.0, scalar2=None,
                    op0=AOT.is_gt)
                nc.sync.dma_start(ag_in[J_OWN:J_OWN + 1, 0:1], la8[:])
                nc.gpsimd.collective_compute(
                    "AllGather", AOT.bypass,
                    ins=[ag_in[:].opt()], outs=[ag_out[:].opt()],
                    replica_groups=[list(range(N_CORES))])
                # counts[p, t] = any_b mask — for the refractory update; runs
                # on DVE during the collective, off the flag critical path
                for t in range(T_TILES):
                    nc.vector.tensor_reduce(
                        out=counts[:, t:t + 1],
                        in_=mask8[:, t * B:(t + 1) * B],
                        axis=mybir.AxisListType.X, op=AOT.max)
                # readback gathered spikes (speculative; only matmul uses it)
                # — split across two DMA queues so the 8 transfers overlap
                for r in range(N_CORES):
                    eng = nc.sync if r % 2 == 0 else nc.scalar
                    eng.dma_start(
                        s_sb[:, r * T_TILES * B:(r + 1) * T_TILES * B]
                        .rearrange("p (kl b) -> p kl b", kl=T_TILES),
                        ag_out[r * (J_OWN + 1): r * (J_OWN + 1) + J_OWN, :]
                        .rearrange("(kl p) b -> p kl b", p=128))
                # gathered flag rows -> global any -> register for next If
                nc.sync.dma_start(
                    fl8[:],
                    ag_out[:].rearrange("(r q) b -> r q b", q=J_OWN + 1)
                    [:, J_OWN:J_OWN + 1, 0:1]
                    .rearrange("r one1 one2 -> one1 (r one2)"))
                nc.vector.tensor_reduce(
                    out=flm[:], in_=fl8[:], axis=mybir.AxisListType.X,
                    op=AOT.max)
                nc.vector.tensor_copy(fli[:], flm[:])
                regs = nc.alloc_registers(f"anyreg{step}")
                nc.regs_load(regs, fli[0:1, 0:1])
                any_sv = nc.snap(regs, donate=True)

                # --- membrane reset + leak: v = v * (mask == 0) * 0.95
                nc.vector.scalar_tensor_tensor(
                    out=v[:], in0=mask8[:], scalar=0.0, in1=v[:],
                    op0=AOT.is_equal, op1=AOT.mult)
                nc.scalar.mul(v[:], v[:], 0.95)

                # --- refractory update (per-neuron [128, T_TILES] vectors)
                nc.vector.tensor_scalar(
                    out=anyv[:], in0=counts[:], scalar1=0.0, scalar2=None,
                    op0=AOT.is_gt)
                nc.vector.copy_predicated(refr[:], anyv[:], three[:])
                nc.vector.tensor_scalar(
                    out=refr[:], in0=refr[:], scalar1=1.0, scalar2=0.0,
                    op0=AOT.subtract, op1=AOT.max)
                nc.vector.tensor_scalar(
                    out=elig[:], in0=refr[:], scalar1=0.0, scalar2=None,
                    op0=AOT.is_equal)

    nc.compile()
    return nc


def _prep_inputs(external_input, connectivity, membrane_potentials,
                 thresholds, refractory_periods):
    """Shard + lay out the full inputs for the 8 per-core NEFF input maps."""
    ext = np.ascontiguousarray(external_input, dtype=np.float32)
    conn = np.ascontiguousarray(connectivity, dtype=np.float32)
    mp = np.asarray(membrane_potentials, dtype=np.float32)
    th = np.asarray(thresholds, dtype=np.float32)
    rf = np.asarray(refractory_periods, dtype=np.float32)

    in_maps = []
    for c in range(N_CORES):
        sl = slice(c * J_OWN, (c + 1) * J_OWN)
        ext_t = np.ascontiguousarray(ext[:, sl].T)               # [512, 1024]
        c_fp8 = np.ascontiguousarray(conn[:, sl]).astype(
            ml_dtypes.float8_e4m3)                               # [4096, 512]
        # [512] -> [128, 4] with n_local = t*128 + p  ->  arr[p, t]
        def vec_tile(x):
            return np.ascontiguousarray(x[sl].reshape(T_TILES, 128).T)
        in_maps.append({
            "ext_t": ext_t,
            "c_fp8": c_fp8,
            "mp": vec_tile(mp),
            "th": vec_tile(th),
            "refr0": vec_tile(rf),
        })
    return in_maps


def kernel(external_input, connectivity, membrane_potentials, thresholds,
           refractory_periods, _trace=False):
    if "nc" not in _CACHE:
        _CACHE["nc"] = build_nc()
    nc = _CACHE["nc"]
    in_maps = _prep_inputs(external_input, connectivity, membrane_potentials,
                           thresholds, refractory_periods)
    res = run_bass_kernel_spmd(nc, in_maps, core_ids=list(range(N_CORES)),
                               trace=_trace)
    _CACHE["last_results"] = res
    out = np.empty((B, N), dtype=np.float32)
    for c in range(N_CORES):
        out[:, c * J_OWN:(c + 1) * J_OWN] = res.results[c]["s_out"].T
    return out

